# revision 5
# baseline (speedup 1.0000x reference)
"""Multi-head attention (B=2, S=2048, D=768, H=12) on 8 Trainium2 NeuronCores.

Sharding: core c -> batch b = c//4, head group g = c%4 (3 heads of 12).
Each core computes, for its batch and its 3 heads:
    Q^T, K^T (features on partitions), V (positions on partitions),
    S^T = K Q^T per 128-row k-block, P~ = exp(S^T/8) (no max subtraction --
    scores are ~N(0,1) so exp cannot overflow), then
    O'^T = [V | 1]^T P~  which yields both the unnormalized output rows and
    the softmax denominator (last row) in one accumulating matmul chain.
    After normalization, the core emits its partial output projection
    out_partial = O_heads @ Wo[head rows]  (no bias).
Host side: inputs are sliced/transposed per core (numpy), outputs are
summed over the 4 head-group partials per batch; bo and the bv term
(softmax rows sum to 1, so V's bias contributes exactly bv @ Wo) are added
on the host.

All matmuls run as float32r (FP22 reduced precision, full PE rate at
moving-dim >= 256); accumulation is fp32 in PSUM.
"""

import sys

import numpy as np

_TRN_REPO = "/opt/trn_rl_repo"
if _TRN_REPO not in sys.path:
    sys.path.insert(0, _TRN_REPO)

import concourse.bacc as bacc
import concourse.mybir as mybir
import concourse.tile as tile
from concourse.bass_utils import run_bass_kernel_spmd

B, S, D, H, HD = 2, 2048, 768, 12, 64
NCORES = 8
HPC = 3  # heads per core
DSL = HPC * HD  # 192: per-core slice of the model dim
KT = D // 128  # 6 contraction tiles for the projections
NKT = S // 128  # 16 key-position blocks
F32 = mybir.dt.float32
F32R = mybir.dt.float32r
AF = mybir.ActivationFunctionType

_cache = {}


def _build():
    nc = bacc.Bacc("TRN2", target_bir_lowering=False, debug=False)

    xq = nc.dram_tensor("xq_t", [D, S], F32R, kind="ExternalInput")
    xk = nc.dram_tensor("xk_t", [D, S], F32R, kind="ExternalInput")
    xv = nc.dram_tensor("xv_t", [D, S], F32R, kind="ExternalInput")
    wq = nc.dram_tensor("wq", [128, KT, DSL], F32R, kind="ExternalInput")
    wk = nc.dram_tensor("wk", [128, KT, DSL], F32R, kind="ExternalInput")
    wv = nc.dram_tensor("wv", [128, KT, DSL], F32R, kind="ExternalInput")
    woa = nc.dram_tensor("wo_a", [128, D], F32R, kind="ExternalInput")
    wob = nc.dram_tensor("wo_b", [64, D], F32R, kind="ExternalInput")
    bqc = nc.dram_tensor("bq_c", [128, 2], F32, kind="ExternalInput")
    ones = nc.dram_tensor("ones_c", [128, NKT * HPC], F32R, kind="ExternalInput")
    outp = nc.dram_tensor("out_p", [S, D], F32, kind="ExternalOutput")

    with tile.TileContext(nc) as tc:
        with (
            tc.tile_pool(name="consts", bufs=1) as consts,
            tc.tile_pool(name="xin", bufs=2) as xin,
            tc.tile_pool(name="acts", bufs=1) as acts,
            tc.tile_pool(name="es", bufs=3) as es,
            tc.tile_pool(name="nrm", bufs=2) as nrm,
            tc.tile_pool(name="outs", bufs=3) as outs,
            tc.tile_pool(name="pp", bufs=2, space="PSUM") as pp,
            tc.tile_pool(name="psn", bufs=2, space="PSUM") as psn,
            tc.tile_pool(name="pon", bufs=1, space="PSUM") as pon,
        ):
            # ---------------- constants ----------------
            wq_sb = consts.tile([128, KT, DSL], F32R)
            nc.sync.dma_start(out=wq_sb[:], in_=wq[:])
            wk_sb = consts.tile([128, KT, DSL], F32R)
            nc.sync.dma_start(out=wk_sb[:], in_=wk[:])
            wv_sb = consts.tile([128, KT, DSL], F32R)
            nc.sync.dma_start(out=wv_sb[:], in_=wv[:])
            woa_sb = consts.tile([128, D], F32R)
            nc.sync.dma_start(out=woa_sb[:], in_=woa[:])
            wob_sb = consts.tile([64, D], F32R)
            nc.sync.dma_start(out=wob_sb[:], in_=wob[:])
            bq_sb = consts.tile([128, 2], F32)
            nc.sync.dma_start(out=bq_sb[:], in_=bqc[:])

            # persistent activations
            qT01 = acts.tile([128, S], F32R)  # heads 0,1 of Q^T
            qT2 = acts.tile([64, S], F32R)  # head 2 of Q^T
            kT01 = acts.tile([128, S], F32R)
            kT2 = acts.tile([64, S], F32R)
            # V with a ones column appended per (k-block, head): [128, 16, 3*65]
            v_sb = acts.tile([128, NKT, HPC * (HD + 1)], F32R)
            nc.sync.dma_start(
                out=v_sb[:].rearrange("p kt (h e) -> p (kt h) e", e=HD + 1)[:, :, HD : HD + 1],
                in_=ones[:],
            )
            oT01 = acts.tile([128, S], F32R)
            oT2 = acts.tile([64, S], F32R)

            xq_r = xq[:].rearrange("(kt p) s -> p kt s", p=128)
            xk_r = xk[:].rearrange("(kt p) s -> p kt s", p=128)
            xv_r = xv[:].rearrange("(kt p) s -> p kt s", p=128)

            # ---------------- projections (streamed in 512-col chunks) ----
            for c in range(4):
                sl = slice(c * 512, (c + 1) * 512)

                # K^T chunk
                xkt = xin.tile([128, KT, 512], F32R, name="x")
                nc.sync.dma_start(out=xkt[:], in_=xk_r[:, :, sl])
                for mt in range(2):
                    m = 128 if mt == 0 else 64
                    pt = pp.tile([128, 512], F32, name="pp")[:m, :]
                    for kt in range(KT):
                        nc.tensor.matmul(
                            pt,
                            lhsT=(wk_sb[:, kt, mt * 128 : mt * 128 + m]),
                            rhs=(xkt[:, kt, :]),
                            start=(kt == 0),
                            stop=(kt == KT - 1),
                        )
                    dst = kT01[:, sl] if mt == 0 else kT2[:, sl]
                    nc.scalar.activation(out=dst, in_=pt, func=AF.Copy)

                # Q^T chunk (+bq via activation bias)
                xqt = xin.tile([128, KT, 512], F32R, name="x")
                nc.sync.dma_start(out=xqt[:], in_=xq_r[:, :, sl])
                for mt in range(2):
                    m = 128 if mt == 0 else 64
                    pt = pp.tile([128, 512], F32, name="pp")[:m, :]
                    for kt in range(KT):
                        nc.tensor.matmul(
                            pt,
                            lhsT=(wq_sb[:, kt, mt * 128 : mt * 128 + m]),
                            rhs=(xqt[:, kt, :]),
                            start=(kt == 0),
                            stop=(kt == KT - 1),
                        )
                    dst = qT01[:, sl] if mt == 0 else qT2[:, sl]
                    nc.scalar.activation(
                        out=dst, in_=pt, func=AF.Identity, bias=bq_sb[:m, mt : mt + 1]
                    )

                # V chunk: V[kpos, hd] with kpos on partitions
                xvt = xin.tile([128, KT, 512], F32R, name="x")
                nc.sync.dma_start(out=xvt[:], in_=xv_r[:, :, sl])
                for i in range(4):
                    pt = pp.tile([128, 512], F32, name="pp")[:, :DSL]
                    for kt in range(KT):
                        nc.tensor.matmul(
                            pt,
                            lhsT=(xvt[:, kt, i * 128 : (i + 1) * 128]),
                            rhs=(wv_sb[:, kt, :]),
                            start=(kt == 0),
                            stop=(kt == KT - 1),
                        )
                    kti = c * 4 + i
                    nc.vector.tensor_copy(
                        out=v_sb[:, kti, :].rearrange("p (h e) -> p h e", h=HPC)[
                            :, :, 0:HD
                        ],
                        in_=pt.rearrange("p (h e) -> p h e", h=HPC),
                    )

            # ---------------- attention ----------------
            for h in range(HPC):
                if h < 2:
                    qT, kTt, base = qT01, kT01, 64 * h
                else:
                    qT, kTt, base = qT2, kT2, 0
                for half in range(2):
                    q0 = half * 1024
                    ot = pon.tile([HD + 1, 1024], F32, name="o")
                    for kt in range(NKT):
                        st = psn.tile([128, 1024], F32, name="s")
                        for j in range(2):
                            nc.tensor.matmul(
                                st[:, j * 512 : (j + 1) * 512],
                                lhsT=(kTt[base : base + 64, kt * 128 : (kt + 1) * 128]),
                                rhs=(qT[base : base + 64, q0 + j * 512 : q0 + (j + 1) * 512]),
                                start=True,
                                stop=True,
                            )
                        et = es.tile([128, 1024], F32R, name="e")
                        nc.scalar.activation(out=et[:], in_=st[:, :], func=AF.Exp, scale=0.125)
                        for j in range(2):
                            nc.tensor.matmul(
                                ot[:, j * 512 : (j + 1) * 512],
                                lhsT=(v_sb[:, kt, h * 65 : (h + 1) * 65]),
                                rhs=(et[:, j * 512 : (j + 1) * 512]),
                                start=(kt == 0),
                                stop=(kt == NKT - 1),
                            )
                    rc = nrm.tile([1, 1024], F32, name="rc")
                    nc.vector.reciprocal(rc[:], ot[HD : HD + 1, :])
                    bc = nrm.tile([64, 1024], F32, name="bc")
                    nc.gpsimd.partition_broadcast(bc[:], rc[:])
                    odst = (
                        oT01[base : base + 64, q0 : q0 + 1024]
                        if h < 2
                        else oT2[:, q0 : q0 + 1024]
                    )
                    nc.vector.tensor_mul(out=odst, in0=ot[0:HD, :], in1=bc[:])

            # ---------------- output projection ----------------
            for qt in range(NKT):
                outt = outs.tile([128, D], F32, name="out")
                qsl = slice(qt * 128, (qt + 1) * 128)
                for ch in range(2):
                    pt = pp.tile([128, 512], F32, name="pp")[:, :384]
                    nc.tensor.matmul(
                        pt,
                        lhsT=(oT01[:, qsl]),
                        rhs=(woa_sb[:, ch * 384 : (ch + 1) * 384]),
                        start=True,
                        stop=False,
                    )
                    nc.tensor.matmul(
                        pt,
                        lhsT=(oT2[:, qsl]),
                        rhs=(wob_sb[:, ch * 384 : (ch + 1) * 384]),
                        start=False,
                        stop=True,
                    )
                    nc.any.tensor_copy(out=outt[:, ch * 384 : (ch + 1) * 384], in_=pt)
                nc.sync.dma_start(out=outp[qsl, :], in_=outt[:])

    nc.compile()
    return nc


def get_nc():
    if "nc" not in _cache:
        _cache["nc"] = _build()
    return _cache["nc"]


def make_in_maps(query, key_, value, Wq, bq, Wk, bk, Wv, bv, Wo, bo):
    """Host-side sharding: per-core input dict (numpy only)."""
    f = np.float32
    query, key_, value = (np.asarray(a, f) for a in (query, key_, value))
    Wq, Wk, Wv, Wo = (np.asarray(a, f) for a in (Wq, Wk, Wv, Wo))
    bq = np.asarray(bq, f)

    in_maps = []
    for c in range(NCORES):
        b, g = c // 4, c % 4
        hsl = slice(g * DSL, (g + 1) * DSL)

        def swz(w):
            # [768, 192] -> [128, 6, 192] with row r = kt*128 + p
            return np.ascontiguousarray(
                w[:, hsl].reshape(KT, 128, DSL).transpose(1, 0, 2)
            )

        bq_c = np.zeros((128, 2), f)
        bq_c[:, 0] = bq[hsl][0:128]
        bq_c[0:64, 1] = bq[hsl][128:DSL]
        in_maps.append(
            {
                "xq_t": np.ascontiguousarray(query[b].T),
                "xk_t": np.ascontiguousarray(key_[b].T),
                "xv_t": np.ascontiguousarray(value[b].T),
                "wq": swz(Wq),
                "wk": swz(Wk),
                "wv": swz(Wv),
                "wo_a": np.ascontiguousarray(Wo[hsl][0:128]),
                "wo_b": np.ascontiguousarray(Wo[hsl][128:DSL]),
                "bq_c": bq_c,
                "ones_c": np.ones((128, NKT * HPC), f),
            }
        )
    return in_maps


def combine(results, Wo, bv, bo):
    """Host-side unshard: sum head-group partials, add bias terms."""
    Wo = np.asarray(Wo, np.float32)
    bv = np.asarray(bv, np.float32)
    bo = np.asarray(bo, np.float32)
    const = (bv @ Wo + bo).astype(np.float32)
    out = np.empty((B, S, D), np.float32)
    for b in range(B):
        acc = results[b * 4]["out_p"].astype(np.float32).copy()
        for g in range(1, 4):
            acc += results[b * 4 + g]["out_p"]
        out[b] = acc + const
    return out


def kernel(query, key_, value, Wq, bq, Wk, bk, Wv, bv, Wo, bo):
    nc = get_nc()
    in_maps = make_in_maps(query, key_, value, Wq, bq, Wk, bk, Wv, bv, Wo, bo)
    res = run_bass_kernel_spmd(nc, in_maps, list(range(NCORES)))
    return combine(res.results, Wo, bv, bo)
